# revision 11
# baseline (speedup 1.0000x reference)
"""ObjCondensationLoss Trainium2 kernel (8 NeuronCores, data-parallel over hits).

Reference semantics (N=100000 hits, K=256 clusters, D=3):
  L_beta = sum(1-beta_ak)/K + (S_b/N_b)*sum(beta[bg])
  q_i    = atanh(beta_i)^2 + q_min
  q_ak   = max_i q_i*M_ik ; x_a = x[argmax] (per cluster)
  L_v    = (1/N) sum_i q_i * sum_k (M*d2 + (1-M)*relu(1-d2)) * q_ak

Identities:
  - q monotone in beta  => one segment-argmax of beta gives beta_ak, q_ak, x_a.
  - v_ik = q_i*q_ak*(1-d2_ik) = h_i . w_k with
      h_i = [q x0, q x1, q x2, q|x|^2, q],  w_k = [2q_k xa, -q_k, q_k(1-|xa|^2)]
  - Lv*N = sum_ik relu(v) + T1 - sum_mem v - sum_mem relu(v)
      T1         = sum_k q_ak * segsum_k(q_i)          (from HS matmul col 4)
      sum_mem v  = sum_kc w_kc * HS_kc,  HS = segsum of h/beta (mask matmul)
      sum_mem relu(v) = per-j masked extract of relu(v) with accum_out.

Schedule vs the original: pass-A masks split ACT (Square+Relu pair, one act
table) / DVE, AllGather triggered right after argmax recovery (~60us, not
~150us), staging transposes hidden in the collective wait, phase-2 matmul in
bf16 (1-pass PE), relu split ACT/DVE, global select via max/max_index.
"""

import numpy as np

N = 100000
K = 256
NC = 8
NLOC = N // NC          # 12500
P = 128
J = 100                 # P*J = 12800 padded local hits
NPAD = P * J
NG = 34                 # transpose groups of 3 hit-cols at bases 0/32/64
Q_MIN = 0.5
S_B = 1.0

# pass-A mask engine split: ACT when (j % 8) < ACT8, else DVE
ACT8 = 5
# phase-2 relu engine split: ACT when (j % 5) < RELU5, else DVE
RELU5 = 4

_CACHE = {}


def _build_nc():
    import concourse.bass as bass
    import concourse.bacc as bacc
    import concourse.mybir as mybir
    import concourse.tile as tile
    from concourse.masks import make_identity

    f32 = mybir.dt.float32
    i32 = mybir.dt.int32
    u32 = mybir.dt.uint32
    bf16 = mybir.dt.bfloat16
    f32r = mybir.dt.float32r
    Alu = mybir.AluOpType
    Act = mybir.ActivationFunctionType
    Ax = mybir.AxisListType

    nc = bacc.Bacc()

    xs = nc.dram_tensor("xs", [P, 3 * J], f32, kind="ExternalInput")
    bs = nc.dram_tensor("bs", [P, J], f32, kind="ExternalInput")
    ys = nc.dram_tensor("ys", [P, J], i32, kind="ExternalInput")
    xf = nc.dram_tensor("xf", [N, 3], f32, kind="ExternalInput")
    out_dr = nc.dram_tensor("out", [1], f32, kind="ExternalOutput")

    spk = nc.dram_tensor("spk", [P, J], f32)        # packed y+beta rows
    ag_in = nc.dram_tensor("ag_in", [2, K], f32)
    ag_out = nc.dram_tensor("ag_out", [2 * NC, K], f32, addr_space="Shared")
    ar_in = nc.dram_tensor("ar_in", [1, 4], f32)
    ar_out = nc.dram_tensor("ar_out", [1, 4], f32, addr_space="Shared")
    RG = [list(range(NC))]

    from contextlib import ExitStack
    with tile.TileContext(nc) as tc, ExitStack() as es:
        cp = es.enter_context(tc.tile_pool(name="cp", bufs=1))
        mk = es.enter_context(tc.tile_pool(name="mk", bufs=8))
        tr = es.enter_context(tc.tile_pool(name="tr", bufs=2))
        pt = es.enter_context(tc.tile_pool(name="pt", bufs=2, space="PSUM"))
        pv = es.enter_context(tc.tile_pool(name="pv", bufs=2, space="PSUM"))
        ph = es.enter_context(tc.tile_pool(name="ph", bufs=1, space="PSUM"))

        V = nc.vector
        S = nc.scalar
        G = nc.gpsimd
        T = nc.tensor
        KT = K // P  # 2 cluster tiles

        # ---------------- constants & input loads ----------------
        identg = cp.tile([P, P], f32)
        make_identity(nc, identg[:])
        ident = cp.tile([P, P], f32)
        V.tensor_copy(ident[:], identg[:])
        ones_col = cp.tile([P, 1], f32)
        V.memset(ones_col[:], 1.0)

        iotaC_i = cp.tile([P, K], i32)
        G.iota(iotaC_i[:], pattern=[[1, K]], base=0, channel_multiplier=0)
        iotaC = cp.tile([P, K], f32)
        V.tensor_copy(iotaC[:], iotaC_i[:])
        iotaC_bf = cp.tile([P, K], bf16)
        V.tensor_copy(iotaC_bf[:], iotaC_i[:])

        jw_i = cp.tile([P, J], i32)          # J - j
        G.iota(jw_i[:], pattern=[[-1, J]], base=J, channel_multiplier=0)
        jw = cp.tile([P, J], f32)
        V.tensor_copy(jw[:], jw_i[:])

        iota8_i = cp.tile([P, NC], i32)
        G.iota(iota8_i[:], pattern=[[1, NC]], base=0, channel_multiplier=0)
        iota8 = cp.tile([P, NC], f32)
        V.tensor_copy(iota8[:], iota8_i[:])
        roff8_i = cp.tile([P, NC], i32)
        G.iota(roff8_i[:], pattern=[[NLOC, NC]], base=0, channel_multiplier=0)
        roff8 = cp.tile([P, NC], f32)
        V.tensor_copy(roff8[:], roff8_i[:])

        x_sb = cp.tile([P, 3 * J], f32)      # (p, j*3+d) interleaved
        nc.sync.dma_start(out=x_sb[:], in_=xs[:])
        beta_sb = cp.tile([P, J], f32)
        nc.sync.dma_start(out=beta_sb[:], in_=bs[:])
        y_i = cp.tile([P, J], i32)
        nc.sync.dma_start(out=y_i[:], in_=ys[:])
        y_f = cp.tile([P, J], f32)
        V.tensor_copy(y_f[:], y_i[:])
        y_bf = cp.tile([P, J], bf16)
        V.tensor_copy(y_bf[:], y_i[:])

        # packed y+beta rows to DRAM for argmax recovery
        spack = cp.tile([P, J], f32)
        V.tensor_tensor(out=spack[:], in0=y_f[:], in1=beta_sb[:], op=Alu.add)
        nc.sync.dma_start(out=spk[:], in_=spack[:])

        # background stats
        bgcol = cp.tile([P, 1], f32)
        tr98 = tr.tile([P, J], f32)
        V.scalar_tensor_tensor(out=tr98[:], in0=y_f[:], scalar=-1.0,
                               in1=beta_sb[:], op0=Alu.is_equal, op1=Alu.mult,
                               accum_out=bgcol[:])
        nbcol = cp.tile([P, 1], f32)
        tr98b = tr.tile([P, J], f32)
        V.tensor_scalar(out=tr98b[:], in0=y_f[:], scalar1=-1.0, scalar2=None,
                        op0=Alu.is_equal, op1=Alu.add, accum_out=nbcol[:])

        # q_i
        lnA = cp.tile([P, J], f32)
        S.activation(lnA[:], beta_sb[:], Act.Ln, bias=1.0, scale=1.0)
        lnB = cp.tile([P, J], f32)
        S.activation(lnB[:], beta_sb[:], Act.Ln, bias=1.0, scale=-1.0)
        ath = cp.tile([P, J], f32)
        V.tensor_tensor(out=ath[:], in0=lnA[:], in1=lnB[:], op=Alu.subtract)
        sq4 = cp.tile([P, J], f32)
        S.activation(sq4[:], ath[:], Act.Square, bias=0.0, scale=0.5)
        validm = cp.tile([P, J], f32)        # y >= -1 (bg included, pads out)
        V.tensor_scalar(out=validm[:], in0=y_f[:], scalar1=-1.5, scalar2=None,
                        op0=Alu.is_gt)
        q0 = cp.tile([P, J], f32)
        V.tensor_scalar(out=q0[:], in0=sq4[:], scalar1=Q_MIN, scalar2=None,
                        op0=Alu.add)
        q_all = cp.tile([P, J], f32)
        V.tensor_tensor(out=q_all[:], in0=q0[:], in1=validm[:], op=Alu.mult)

        x0 = cp.tile([P, J], f32)
        x1 = cp.tile([P, J], f32)
        x2 = cp.tile([P, J], f32)
        V.tensor_copy(x0[:], x_sb[:, 0:3 * J:3])
        V.tensor_copy(x1[:], x_sb[:, 1:3 * J:3])
        V.tensor_copy(x2[:], x_sb[:, 2:3 * J:3])
        sqn = cp.tile([P, J], f32)
        tmpb = cp.tile([P, J], f32)
        V.tensor_tensor(out=sqn[:], in0=x0[:], in1=x0[:], op=Alu.mult)
        V.tensor_tensor(out=tmpb[:], in0=x1[:], in1=x1[:], op=Alu.mult)
        V.tensor_tensor(out=sqn[:], in0=sqn[:], in1=tmpb[:], op=Alu.add)
        V.tensor_tensor(out=tmpb[:], in0=x2[:], in1=x2[:], op=Alu.mult)
        V.tensor_tensor(out=sqn[:], in0=sqn[:], in1=tmpb[:], op=Alu.add)

        # h/beta staging for pass-A segment-sum matmuls: (h/b)*(b*onehot)
        bsafe = cp.tile([P, J], f32)
        V.tensor_scalar(out=bsafe[:], in0=validm[:], scalar1=-1.0, scalar2=1.0,
                        op0=Alu.mult, op1=Alu.add)      # 1 for pads, 0 valid
        V.tensor_tensor(out=bsafe[:], in0=bsafe[:], in1=beta_sb[:], op=Alu.add)
        binv = cp.tile([P, J], f32)
        V.reciprocal(out=binv[:], in_=bsafe[:])
        qob = cp.tile([P, J], f32)
        V.tensor_tensor(out=qob[:], in0=q_all[:], in1=binv[:], op=Alu.mult)
        qobv = qob[:].rearrange("p (j o) -> p j o", o=1)
        staging2 = cp.tile([P, 5 * J], f32r)
        st2 = staging2[:].rearrange("p (j c) -> p j c", c=5)
        V.tensor_tensor(out=st2[:, :, 0:1],
                        in0=x0[:].rearrange("p (j o) -> p j o", o=1),
                        in1=qobv, op=Alu.mult)
        V.tensor_tensor(out=st2[:, :, 1:2],
                        in0=x1[:].rearrange("p (j o) -> p j o", o=1),
                        in1=qobv, op=Alu.mult)
        V.tensor_tensor(out=st2[:, :, 2:3],
                        in0=x2[:].rearrange("p (j o) -> p j o", o=1),
                        in1=qobv, op=Alu.mult)
        V.tensor_tensor(out=st2[:, :, 3:4],
                        in0=sqn[:].rearrange("p (j o) -> p j o", o=1),
                        in1=qobv, op=Alu.mult)
        V.tensor_copy(st2[:, :, 4:5], qobv)

        # ---------------- pass A: segment max + segment sum ----------------
        negy = cp.tile([P, J], f32)
        V.tensor_scalar(out=negy[:], in0=y_f[:], scalar1=-1.0, scalar2=None,
                        op0=Alu.mult)
        negb = cp.tile([P, J], f32)
        V.tensor_scalar(out=negb[:], in0=beta_sb[:], scalar1=-1.0, scalar2=None,
                        op0=Alu.mult)
        accA = cp.tile([P, K], f32)
        V.memset(accA[:], 0.0)
        HST = ph.tile([5, K], f32, tag="HST")
        for j in range(J):
            if j % 8 < ACT8:
                # t=(iota-y)^2 then beta*relu(1-t): exact 0/beta mask
                tq = mk.tile([P, K], f32, tag=f"tq{j % 2}")
                S.activation(tq[:], iotaC[:], Act.Square,
                             bias=negy[:, j:j + 1], scale=1.0)
                m2 = mk.tile([P, K], f32r, tag=f"ma{j % 2}")
                S.activation(m2[:], tq[:], Act.Relu,
                             bias=beta_sb[:, j:j + 1], scale=negb[:, j:j + 1])
            else:
                m2 = mk.tile([P, K], f32r, tag=f"md{j % 2}")
                V.tensor_scalar(out=m2[:], in0=iotaC[:],
                                scalar1=y_f[:, j:j + 1],
                                scalar2=beta_sb[:, j:j + 1],
                                op0=Alu.is_equal, op1=Alu.mult)
            T.matmul(out=HST[:], lhsT=st2[:, j, 0:5], rhs=m2[:],
                     start=(j == 0), stop=(j == J - 1), skip_group_check=True)
            V.tensor_tensor(out=accA[:], in0=accA[:], in1=m2[:], op=Alu.max)

        # ---------------- argmax recovery + AllGather trigger ----------------
        beta_loc, glx = [], []
        for kt in range(KT):
            pT = pt.tile([P, P], f32, tag="pT")
            T.transpose(out=pT[:], in_=accA[:, kt * P:(kt + 1) * P],
                        identity=ident[:])
            accT = cp.tile([P, P], f32, tag=f"accT{kt}")
            S.copy(accT[:], pT[:])
            top8 = cp.tile([P, 8], f32, tag=f"top8{kt}")
            V.max(top8[:], accT[:])
            idx8 = cp.tile([P, 8], u32, tag=f"idx8{kt}")
            V.max_index(idx8[:], top8[:], accT[:])
            bl = cp.tile([P, 1], f32, tag=f"bl{kt}")
            V.tensor_copy(bl[:], top8[:, 0:1])
            beta_loc.append(bl)
            psI = cp.tile([P, 1], i32, tag=f"psI{kt}")
            V.tensor_copy(psI[:], idx8[:, 0:1])
            psF = cp.tile([P, 1], f32, tag=f"psF{kt}")
            V.tensor_copy(psF[:], idx8[:, 0:1])

            SR = cp.tile([P, J], f32, tag=f"SR{kt}")
            G.indirect_dma_start(
                out=SR[:], out_offset=None, in_=spk[:],
                in_offset=bass.IndirectOffsetOnAxis(ap=psI[:, 0:1], axis=0))

            kv_i = cp.tile([P, 1], i32, tag=f"kvi{kt}")
            G.iota(kv_i[:], pattern=[[1, 1]], base=kt * P, channel_multiplier=1)
            kv = cp.tile([P, 1], f32, tag=f"kv{kt}")
            V.tensor_copy(kv[:], kv_i[:])
            sstar = cp.tile([P, 1], f32, tag=f"ss{kt}")
            V.tensor_tensor(out=sstar[:], in0=kv[:], in1=bl[:], op=Alu.add)
            w = cp.tile([P, J], f32, tag=f"w{kt}")
            V.scalar_tensor_tensor(out=w[:], in0=SR[:], scalar=sstar[:, 0:1],
                                   in1=jw[:], op0=Alu.is_equal, op1=Alu.mult)
            vmax = cp.tile([P, 1], f32, tag=f"vm{kt}")
            V.reduce_max(out=vmax[:], in_=w[:], axis=Ax.X)
            jst = cp.tile([P, 1], f32, tag=f"jst{kt}")
            V.tensor_scalar(out=jst[:], in0=vmax[:], scalar1=-1.0,
                            scalar2=float(J), op0=Alu.mult, op1=Alu.add)
            valid = cp.tile([P, 1], f32, tag=f"vd{kt}")
            V.tensor_scalar(out=valid[:], in0=bl[:], scalar1=0.0, scalar2=None,
                            op0=Alu.is_gt)
            glin = cp.tile([P, 1], f32, tag=f"gl{kt}")
            V.scalar_tensor_tensor(out=glin[:], in0=psF[:], scalar=float(J),
                                   in1=jst[:], op0=Alu.mult, op1=Alu.add)
            gl = cp.tile([P, 1], f32, tag=f"glv{kt}")
            V.tensor_tensor(out=gl[:], in0=glin[:], in1=valid[:], op=Alu.mult)
            glx.append(gl)

        sb4 = cp.tile([P, 4], f32)
        V.tensor_copy(sb4[:, 0:1], beta_loc[0][:])
        V.tensor_copy(sb4[:, 1:2], beta_loc[1][:])
        V.tensor_copy(sb4[:, 2:3], glx[0][:])
        V.tensor_copy(sb4[:, 3:4], glx[1][:])
        nc.sync.dma_start(out=ag_in[:].rearrange("a (c p) -> p (a c)", p=P),
                          in_=sb4[:])
        G.collective_compute("AllGather", mybir.AluOpType.bypass,
                             replica_groups=RG, ins=[ag_in[:]], outs=[ag_out[:]])

        # ---------------- collective-wait window work ----------------
        # staging (p, g*96 + a*32 + c): h columns c=0..4 for hit j=g*3+a
        staging = cp.tile([P, NG * 96], f32)
        V.memset(staging[:], 0.0)
        stj = staging[:].rearrange("p (j s) -> p j s", s=32)
        for c, src in ((0, x0), (1, x1), (2, x2), (3, sqn)):
            V.tensor_tensor(out=stj[:, 0:J, c:c + 1],
                            in0=src[:].rearrange("p (j o) -> p j o", o=1),
                            in1=q_all[:].rearrange("p (j o) -> p j o", o=1),
                            op=Alu.mult)
        V.tensor_copy(stj[:, 0:J, 4:5],
                      q_all[:].rearrange("p (j o) -> p j o", o=1))

        hts = []
        for g in range(NG):
            pT = pt.tile([96, P], f32, tag="pT")
            T.transpose(out=pT[:], in_=staging[:, g * 96:(g + 1) * 96],
                        identity=ident[:])
            hT = cp.tile([96, P], bf16, tag=f"hT{g}")
            if g % 2 == 0:
                S.copy(hT[:], pT[:])
            else:
                V.tensor_copy(hT[:], pT[:])
            hts.append(hT)

        # ---------------- post-AllGather: global select ----------------
        bsrc = cp.tile([NC, K], f32)
        nc.sync.dma_start(out=bsrc[:], in_=ag_out[0:2 * NC:2, :])
        gsrc = cp.tile([NC, K], f32)
        nc.sync.dma_start(out=gsrc[:], in_=ag_out[1:2 * NC:2, :])

        bg2 = cp.tile([P, KT], f32)      # global beta_ak, col per ktile
        gi2 = cp.tile([P, KT], f32)      # global hit index of argmax
        for kt in range(KT):
            pT = ph.tile([P, NC], f32, tag="pTs")
            T.transpose(out=pT[:], in_=bsrc[:, kt * P:(kt + 1) * P],
                        identity=ident[0:NC, 0:NC])
            bt = cp.tile([P, NC], f32, tag=f"bt{kt}")
            S.copy(bt[:], pT[:])
            pT2 = ph.tile([P, NC], f32, tag="pTs")
            T.transpose(out=pT2[:], in_=gsrc[:, kt * P:(kt + 1) * P],
                        identity=ident[0:NC, 0:NC])
            gt = cp.tile([P, NC], f32, tag=f"gt{kt}")
            S.copy(gt[:], pT2[:])
            gtg = cp.tile([P, NC], f32, tag=f"gtg{kt}")
            V.tensor_tensor(out=gtg[:], in0=gt[:], in1=roff8[:], op=Alu.add)
            t8 = cp.tile([P, 8], f32, tag=f"t8{kt}")
            V.max(t8[:], bt[:])
            i8 = cp.tile([P, 8], u32, tag=f"i8{kt}")
            V.max_index(i8[:], t8[:], bt[:])
            V.tensor_copy(bg2[:, kt:kt + 1], t8[:, 0:1])
            rf = cp.tile([P, 1], f32, tag=f"rf{kt}")
            V.tensor_copy(rf[:], i8[:, 0:1])
            tsel = tr.tile([P, NC], f32, tag="tsel")
            V.scalar_tensor_tensor(out=tsel[:], in0=iota8[:], scalar=rf[:, 0:1],
                                   in1=gtg[:], op0=Alu.is_equal, op1=Alu.mult,
                                   accum_out=gi2[:, kt:kt + 1])

        om2 = cp.tile([P, KT], f32)      # 0 for empty clusters
        V.tensor_scalar(out=om2[:], in0=bg2[:], scalar1=0.0, scalar2=None,
                        op0=Alu.is_gt)
        gi2m = cp.tile([P, KT], f32)
        V.tensor_tensor(out=gi2m[:], in0=gi2[:], in1=om2[:], op=Alu.mult)
        giI = cp.tile([P, KT], i32)
        V.tensor_copy(giI[:], gi2m[:])

        # q_ak
        la2 = cp.tile([P, KT], f32)
        S.activation(la2[:], bg2[:], Act.Ln, bias=1.0, scale=1.0)
        lb2 = cp.tile([P, KT], f32)
        S.activation(lb2[:], bg2[:], Act.Ln, bias=1.0, scale=-1.0)
        at2 = cp.tile([P, KT], f32)
        V.tensor_tensor(out=at2[:], in0=la2[:], in1=lb2[:], op=Alu.subtract)
        s42 = cp.tile([P, KT], f32)
        S.activation(s42[:], at2[:], Act.Square, bias=0.0, scale=0.5)
        qa0 = cp.tile([P, KT], f32)
        V.tensor_scalar(out=qa0[:], in0=s42[:], scalar1=Q_MIN, scalar2=None,
                        op0=Alu.add)
        qa2 = cp.tile([P, KT], f32)
        V.tensor_tensor(out=qa2[:], in0=qa0[:], in1=om2[:], op=Alu.mult)

        # x_a gathers (one [P,1] row-gather per ktile)
        xa2 = cp.tile([P, KT * 3], f32)
        for kt in range(KT):
            G.indirect_dma_start(
                out=xa2[:, kt * 3:kt * 3 + 3], out_offset=None, in_=xf[:],
                in_offset=bass.IndirectOffsetOnAxis(ap=giI[:, kt:kt + 1],
                                                    axis=0))

        # cluster weight rows Wt [P, kt, 0:5] = [2 qa xa, -qa, qa(1-|xa|^2)]
        xx2 = cp.tile([P, KT * 3], f32)
        V.tensor_tensor(out=xx2[:], in0=xa2[:], in1=xa2[:], op=Alu.mult)
        sn2 = cp.tile([P, KT], f32)
        V.reduce_sum(out=sn2[:].rearrange("p (a o) -> p a o", o=1),
                     in_=xx2[:].rearrange("p (a c) -> p a c", a=KT), axis=Ax.X)
        q22 = cp.tile([P, KT], f32)
        V.tensor_scalar(out=q22[:], in0=qa2[:], scalar1=2.0, scalar2=None,
                        op0=Alu.mult)
        t1m2 = cp.tile([P, KT], f32)
        V.scalar_tensor_tensor(out=t1m2[:], in0=sn2[:], scalar=-1.0,
                               in1=qa2[:], op0=Alu.mult, op1=Alu.mult)
        wt8 = cp.tile([P, KT * 8], f32)
        w8v = wt8[:].rearrange("p (a c) -> p a c", a=KT)
        V.tensor_tensor(out=w8v[:, :, 0:3],
                        in0=xa2[:].rearrange("p (a c) -> p a c", a=KT),
                        in1=q22[:].rearrange("p (a o) -> p a o", o=1).to_broadcast([P, KT, 3]),
                        op=Alu.mult)
        V.tensor_scalar(out=w8v[:, :, 3:4].rearrange("p a o -> p (a o)"),
                        in0=qa2[:], scalar1=-1.0, scalar2=None, op0=Alu.mult)
        V.tensor_tensor(out=w8v[:, :, 4:5].rearrange("p a o -> p (a o)"),
                        in0=qa2[:], in1=t1m2[:], op=Alu.add)

        # wall3x: transposed cluster weights replicated at bases 0/32/64
        wallp = ph.tile([5, K], f32, tag="wallp")
        for kt in range(KT):
            T.transpose(out=wallp[:, kt * P:(kt + 1) * P],
                        in_=w8v[:, kt, 0:5], identity=ident[:])
        wall3x = cp.tile([69, K], bf16)
        for b in (0, 32, 64):
            S.copy(wall3x[b:b + 5, :], wallp[:])

        # HS reductions (off the tail: HST is complete since pass A)
        SUM = cp.tile([P, 10], f32)
        V.memset(SUM[:], 0.0)
        hs_sb = cp.tile([5, K], f32)
        S.copy(hs_sb[:], HST[:])
        for kt in range(KT):
            pT = ph.tile([P, 5], f32, tag="pTs")
            T.transpose(out=pT[:], in_=hs_sb[:, kt * P:(kt + 1) * P],
                        identity=ident[0:5, 0:5])
            hst_t = cp.tile([P, 5], f32, tag=f"hstt{kt}")
            S.copy(hst_t[:], pT[:])
            # T1 col: q_ak * segsum_q
            V.tensor_tensor(out=SUM[:, 2 + kt:3 + kt],
                            in0=qa2[:, kt:kt + 1], in1=hst_t[:, 4:5],
                            op=Alu.mult)
            # -sum_mem v col: -dot(W_k, HS_k)
            wdot = cp.tile([P, 5], f32, tag=f"wdot{kt}")
            V.tensor_tensor(out=wdot[:], in0=w8v[:, kt, 0:5], in1=hst_t[:],
                            op=Alu.mult)
            V.reduce_sum(out=SUM[:, 4 + kt:5 + kt], in_=wdot[:], axis=Ax.X,
                         negate=True)
            # sbet col: (1 - beta_g)
            V.tensor_scalar(out=SUM[:, 8 + kt:9 + kt],
                            in0=bg2[:, kt:kt + 1], scalar1=-1.0, scalar2=1.0,
                            op0=Alu.mult, op1=Alu.add)
        V.tensor_copy(SUM[:, 6:7], bgcol[:])
        V.tensor_copy(SUM[:, 7:8], nbcol[:])

        # ---------------- phase 2: potential matmul + relu + extract --------
        rcol = cp.tile([P, J], f32)
        mcol = cp.tile([P, J], f32)
        for j in range(J):
            g, a = j // 3, j % 3
            pvt = pv.tile([P, K], f32, tag="pvt")
            T.matmul(out=pvt[:], lhsT=hts[g][32 * a:32 * a + 5, :],
                     rhs=wall3x[32 * a:32 * a + 5, :], start=True, stop=True,
                     skip_group_check=True)
            ta = tr.tile([P, K], bf16, tag=f"ta{j % 2}")
            if j % 5 < RELU5:
                S.activation(ta[:], pvt[:], Act.Relu,
                             accum_out=rcol[:, j:j + 1])
            else:
                V.tensor_scalar(out=ta[:], in0=pvt[:], scalar1=0.0,
                                scalar2=None, op0=Alu.max, op1=Alu.add,
                                accum_out=rcol[:, j:j + 1])
            td = tr.tile([P, K], bf16, tag=f"td{j % 2}")
            V.scalar_tensor_tensor(out=td[:], in0=iotaC_bf[:],
                                   scalar=y_bf[:, j:j + 1], in1=ta[:],
                                   op0=Alu.is_equal, op1=Alu.mult,
                                   accum_out=mcol[:, j:j + 1])

        # ---------------- reductions & loss ----------------
        V.reduce_sum(out=SUM[:, 0:1], in_=rcol[:], axis=Ax.X)
        V.reduce_sum(out=SUM[:, 1:2], in_=mcol[:], axis=Ax.X, negate=True)

        SUMa = cp.tile([P, 10], f32)
        S.copy(SUMa[:], SUM[:])
        sump = ph.tile([1, 10], f32, tag="sump")
        T.matmul(out=sump[:], lhsT=ones_col[:], rhs=SUMa[:], start=True,
                 stop=True)
        sums = cp.tile([1, 10], f32)
        S.copy(sums[:], sump[:])

        # lv_loc = r - mrelu + T1(0) + T1(1) - mv0 - mv1 (cols 1,4,5 negated)
        lv = cp.tile([1, 1], f32)
        V.reduce_sum(out=lv[:], in_=sums[0:1, 0:6], axis=Ax.X)

        arp = cp.tile([1, 4], f32)
        G.memset(arp[:], 0.0)
        V.tensor_copy(arp[:, 0:1], lv[:])
        V.tensor_copy(arp[:, 1:2], sums[0:1, 6:7])
        V.tensor_copy(arp[:, 2:3], sums[0:1, 7:8])
        nc.sync.dma_start(out=ar_in[:], in_=arp[:])
        G.collective_compute("AllReduce", mybir.AluOpType.add,
                             replica_groups=RG, ins=[ar_in[:]], outs=[ar_out[:]])
        ars = cp.tile([1, 4], f32)
        nc.sync.dma_start(out=ars[:], in_=ar_out[:])

        nbinv = cp.tile([1, 1], f32)
        V.reciprocal(out=nbinv[:], in_=ars[:, 2:3])
        tbg = cp.tile([1, 1], f32)
        V.tensor_tensor(out=tbg[:], in0=ars[:, 1:2], in1=nbinv[:], op=Alu.mult)
        V.tensor_scalar(out=tbg[:], in0=tbg[:], scalar1=float(S_B),
                        scalar2=None, op0=Alu.mult)
        sbet = cp.tile([1, 1], f32)
        V.tensor_tensor(out=sbet[:], in0=sums[0:1, 8:9], in1=sums[0:1, 9:10],
                        op=Alu.add)
        loss = cp.tile([1, 1], f32)
        V.tensor_scalar(out=loss[:], in0=sbet[:], scalar1=1.0 / K,
                        scalar2=None, op0=Alu.mult)
        V.tensor_tensor(out=loss[:], in0=loss[:], in1=tbg[:], op=Alu.add)
        tlv = cp.tile([1, 1], f32)
        V.tensor_scalar(out=tlv[:], in0=ars[:, 0:1],
                        scalar1=float(np.float32(1.0 / N)), scalar2=None,
                        op0=Alu.mult)
        V.tensor_tensor(out=loss[:], in0=loss[:], in1=tlv[:], op=Alu.add)
        nc.sync.dma_start(out=out_dr[None, :], in_=loss[:])

    if not nc.is_finalized():
        nc.finalize()
    return nc


def _shard_inputs(x, beta, y):
    x = np.ascontiguousarray(np.asarray(x, dtype=np.float32))
    beta = np.ascontiguousarray(np.asarray(beta, dtype=np.float32))
    y = np.ascontiguousarray(np.asarray(y)).astype(np.int32)
    in_maps = []
    for r in range(NC):
        sl = slice(r * NLOC, (r + 1) * NLOC)
        xp = np.zeros((NPAD, 3), np.float32)
        bp = np.zeros((NPAD,), np.float32)
        yp = np.full((NPAD,), -2, np.int32)
        xp[:NLOC] = x[sl]
        bp[:NLOC] = beta[sl]
        yp[:NLOC] = y[sl]
        in_maps.append({
            "xs": xp.reshape(P, 3 * J),
            "bs": bp.reshape(P, J),
            "ys": yp.reshape(P, J),
            "xf": x,
        })
    return in_maps


def _install_ntff_hook_shim():
    """antenv.axon_hooks is absent in this image; recreate it via ctypes
    so run_bass_kernel_spmd(trace=True) can capture NTFF profiles."""
    import sys
    import types
    try:
        import antenv.axon_hooks  # noqa: F401
        return
    except ImportError:
        pass
    try:
        import antenv
        from trn_agent_boot.trn_boot import _ntff_profile_via_ctypes
        hook = _ntff_profile_via_ctypes("/opt/axon/libaxon_pjrt.so")
        mod = types.ModuleType("antenv.axon_hooks")
        mod._hook = hook
        mod.get_axon_ntff_profile_hook = lambda: mod._hook
        mod.set_axon_ntff_profile_hook = lambda h: setattr(mod, "_hook", h)
        sys.modules["antenv.axon_hooks"] = mod
        antenv.axon_hooks = mod
    except Exception as e:  # degrade to no tracing
        print(f"ntff hook shim failed: {e}")


def kernel(x, beta, y, K=256, S_b=1.0, q_min=0.5):
    import os
    assert int(K) == 256 and float(S_b) == 1.0 and float(q_min) == 0.5
    if int(os.environ.get("KERNEL_TRACE", "0")):
        _install_ntff_hook_shim()
    if "nc" not in _CACHE:
        _CACHE["nc"] = _build_nc()
    from concourse.bass_utils import run_bass_kernel_spmd
    in_maps = _shard_inputs(x, beta, y)
    trace = bool(int(os.environ.get("KERNEL_TRACE", "0")))
    res = run_bass_kernel_spmd(_CACHE["nc"], in_maps, core_ids=list(range(NC)),
                               trace=trace)
    _CACHE["last_results"] = res
    return np.float32(np.asarray(res.results[0]["out"]).reshape(-1)[0])


def run_sim(x, beta, y):
    """Multi-core simulator run (no hardware)."""
    import concourse.bass_interp as bass_interp
    if "nc" not in _CACHE:
        _CACHE["nc"] = _build_nc()
    nc = _CACHE["nc"]
    in_maps = _shard_inputs(x, beta, y)
    sim = bass_interp.MultiCoreSim(nc, NC)
    for r in range(NC):
        for k, v in in_maps[r].items():
            sim.cores[r].tensor(k)[:] = v
    sim.simulate()
    return np.float32(np.asarray(sim.cores[0].mem_tensor("out")).reshape(-1)[0])


if __name__ == "__main__":
    import sys
    sys.path.insert(0, "/root/problem")
    import jax
    import reference
    with jax.default_device(jax.devices("cpu")[0]):
        inputs = reference.setup_inputs()
        inputs = {k: (np.asarray(v) if hasattr(v, "shape") else v)
                  for k, v in inputs.items()}
        expected = float(reference.reference(**inputs))
    if "--sim" in sys.argv:
        got = float(run_sim(inputs["x"], inputs["beta"], inputs["y"]))
    else:
        got = float(kernel(**{k: (np.asarray(v) if hasattr(v, "shape") else v)
                              for k, v in inputs.items()}))
    rel = abs(got - expected) / max(abs(expected), 1e-30)
    print(f"expected={expected!r} got={got!r} rel={rel:.3e}")
